# revision 1
# baseline (speedup 1.0000x reference)
"""Trainium2 Bass kernel for Clique2NodeConvBasic (GNN message passing).

Computes, for the fixed problem size N=100000 nodes, C=50000 cliques,
E=1600000 edges, D=128:

    gathered = x_clique[clique_idx]            # [E, 128]
    summed   = segment_sum(gathered, node_idx) # [N, 128]
    mean     = summed / max(count, 1)
    out      = mean @ W.T + b                  # [N, 128]

Sharding: edges are partitioned by destination-node range across the 8
NeuronCores (nodes 12500 per core); x_clique and the 128x128 Linear are
replicated. Segment-sum applies locally, no cross-device reduction.

Per-core device algorithm:
  - host sorts edges by destination and buckets them into 98 blocks of
    128 destination nodes; each block's edge list is split by clique id
    at 32768 (dma_gather indices are int16) and padded to a fixed number
    of 128-edge tiles (T_A / T_B, global constants derived from data)
  - dma_gather batches fetch the x_clique rows for a group of blocks
  - a one-hot matrix (edge -> node-within-block) is built with a single
    batched DVE is_equal against an iota tile
  - PE accumulates accum[f, n] += G[e, f].T @ onehot[e, n] in PSUM; the
    gathered tile must be the STATIONARY operand — the PE's moving-operand
    path crashes when streaming a dma_gather-written tile
  - epilogue per block: ACT copies PSUM->SBUF, one matmul with W.T applies
    the Linear directly on the [f, n] accumulator (no transpose needed),
    ACT scales by 1/count (host-precomputed, per-partition = per-node),
    DVE adds the broadcast bias, and the [128, 128] rows are DMA'd out.

Measured on 8 axon NeuronCores: ~1.95-1.97 ms HW exec. The bottleneck is
GpSimd Q7 descriptor generation inside dma_gather (~8 ns per gathered
row, 102%% engine occupancy); HBM, PE, DVE and ACT all run well below
30%% occupancy underneath it.
"""

import os
import sys
import types

sys.path.insert(0, "/opt/trn_rl_repo")

import numpy as np

import concourse.bass as bass
import concourse.mybir as mybir
import concourse.tile as tile
from concourse.vector_clock import ScopedClock, VectorClock
from concourse.bass_utils import run_bass_kernel_spmd

# ----------------------------------------------------------------------------
# Environment shims
# ----------------------------------------------------------------------------

def _install_ntff_shim():
    """Register the axon NTFF profile hook if the image's antenv lacks it."""
    try:
        import antenv
    except ImportError:
        return
    if hasattr(antenv, "axon_hooks"):
        return
    hooks_mod = types.ModuleType("antenv.axon_hooks")
    _store = [None]
    hooks_mod.set_axon_ntff_profile_hook = lambda h: _store.__setitem__(0, h)
    hooks_mod.get_axon_ntff_profile_hook = lambda: _store[0]
    sys.modules["antenv.axon_hooks"] = hooks_mod
    antenv.axon_hooks = hooks_mod
    try:
        from trn_agent_boot.trn_boot import _ntff_profile_via_ctypes

        hook = _ntff_profile_via_ctypes("/opt/axon/libaxon_pjrt.so")
        if hook is not None:
            hooks_mod.set_axon_ntff_profile_hook(hook)
    except Exception:
        pass


_install_ntff_shim()


class PatchedTileContext(tile.TileContext):
    """Spread the tail-drain's sem waits over a chain of SP NOPs.

    The walrus build in this container caps sync-waits per instruction
    (setupSyncWait: "Too many sync wait commands"), while stock Tile
    attaches every outstanding proc's wait to one Drain. One NOP per
    proc keeps every instruction at a single wait.
    """

    def _drain_and_barrier(self, tick_clock, wait_clock):
        gc = tick_clock.global_clock
        for p, t in enumerate(gc):
            if t <= 0:
                continue
            nop = self.nc.sync.nop()
            part = VectorClock()
            part.require_at_least(p, t)
            wait_clock.add_sem_waits(nop.ins, ScopedClock({None: part}))
        self.nc.sync.drain()
        self.nc.all_engine_barrier()
        assert self.sems is not None
        popped = self.nc._tile_sem_poison_stack.pop()
        assert popped is self._sem_poison
        self.nc.clear_and_free_semaphores(list(self.sems.allocated().values()))
        self.nc.all_engine_barrier()


# ----------------------------------------------------------------------------
# Problem constants (hardcoded per the task contract)
# ----------------------------------------------------------------------------

N_NODES = 100000
N_CLIQUES = 50000
D = 128
N_CORES = 8
NPC = N_NODES // N_CORES        # 12500 nodes per core
BLK = 128                       # destination nodes per block
NBLK = -(-NPC // BLK)           # 98 blocks per core (last partial: 84)
NPAD = NBLK * BLK               # 12544 padded output rows per core
SPLIT = 32768                   # int16-index limit for dma_gather
GRP = 2                         # blocks gathered per dma_gather call
PAD_DEST = -1000.0              # one-hot miss value for padding slots

# bf16 halves gather bytes but the kernel is GpSimd-descriptor-bound, so it
# is no faster (1.95ms vs 1.97ms) and costs 4 decades of accuracy. Default f32.
USE_BF16 = os.environ.get("KERNEL_BF16", "0") == "1"

_F32 = mybir.dt.float32
_DT = mybir.dt.bfloat16 if USE_BF16 else _F32
_NP_DT = np.dtype("bfloat16") if False else None  # numpy lacks bf16; use ml_dtypes

if USE_BF16:
    import ml_dtypes

    _NP_DT = np.dtype(ml_dtypes.bfloat16)
else:
    _NP_DT = np.dtype(np.float32)


# ----------------------------------------------------------------------------
# Host-side preparation
# ----------------------------------------------------------------------------

def _prepare(x_clique, node2clique_index):
    """Sort/bucket/pad the edge list. Returns per-core input dicts plus the
    (data-dependent) tile counts T_A, T_B."""
    node = np.asarray(node2clique_index[0]).astype(np.int64)
    clique = np.asarray(node2clique_index[1]).astype(np.int64)

    counts = np.bincount(node, minlength=N_NODES).astype(np.float64)
    inv_cnt = (1.0 / np.maximum(counts, 1.0)).astype(np.float32)

    order = np.argsort(node, kind="stable")
    ns = node[order]
    cs = clique[order]

    core_bounds = np.searchsorted(ns, np.arange(N_CORES + 1) * NPC)

    # First pass: per-(core, block) A/B counts to fix the global T_A, T_B.
    per_core = []
    maxA = 0
    maxB = 0
    for c in range(N_CORES):
        lo, hi = core_bounds[c], core_bounds[c + 1]
        loc = ns[lo:hi] - c * NPC
        cq = cs[lo:hi]
        blk = loc // BLK
        win = loc % BLK
        is_a = cq < SPLIT
        # edges already sorted by loc; stable-partition A before B per block
        key = blk * 2 + (~is_a)
        sub = np.argsort(key, kind="stable")
        blk, win, cq, is_a = blk[sub], win[sub], cq[sub], is_a[sub]
        cntA = np.bincount(blk[is_a], minlength=NBLK)
        cntB = np.bincount(blk[~is_a], minlength=NBLK)
        maxA = max(maxA, int(cntA.max()))
        maxB = max(maxB, int(cntB.max()))
        per_core.append((blk, win, cq, is_a, cntA, cntB))

    T_A = -(-maxA // 128)
    T_B = max(-(-maxB // 128), 1)
    T = T_A + T_B
    LA = T_A * 128
    LB = T_B * 128
    NGRP = -(-NBLK // GRP)      # 49
    np_dt = _NP_DT

    in_maps = []
    for c in range(N_CORES):
        blk, win, cq, is_a, cntA, cntB = per_core[c]

        idxA = np.zeros((NBLK, LA), dtype=np.int16)
        idxB = np.zeros((NBLK, LB), dtype=np.int16)
        dest = np.full((NBLK, T * 128), PAD_DEST, dtype=np.float32)

        offA = np.concatenate([[0], np.cumsum(cntA)])
        offB = np.concatenate([[0], np.cumsum(cntB)])
        nA_tot = int(offA[-1])
        winA = win[: nA_tot] if False else None  # placeholder, replaced below

        # A edges come first within each block (stable partition above)
        a_idx = np.flatnonzero(is_a)
        b_idx = np.flatnonzero(~is_a)
        cqA, winA, blkA = cq[a_idx], win[a_idx], blk[a_idx]
        cqB, winB, blkB = cq[b_idx] - SPLIT, win[b_idx], blk[b_idx]

        posA = np.arange(len(a_idx)) - offA[blkA]
        posB = np.arange(len(b_idx)) - offB[blkB]
        idxA[blkA, posA] = cqA.astype(np.int16)
        idxB[blkB, posB] = cqB.astype(np.int16)
        dest[blkA, posA] = winA
        dest[blkB, posB + LA] = winB


        # wrap indices for dma_gather: seq j -> [j % 16, j // 16], grouped
        # GRP blocks per gather call, concatenated along the free dim.
        # dma_gather reads a [128, n/16] idx AP: the [16, n/16] wrap is
        # replicated across all 8 GpSimd cores' partition groups.
        def _wrap(idx, L):
            w = idx.reshape(NGRP, GRP * L).reshape(NGRP, -1, 16)
            w = np.ascontiguousarray(np.transpose(w, (2, 0, 1))).reshape(16, -1)
            return np.tile(w, (8, 1))

        wA = _wrap(idxA, LA)
        wB = _wrap(idxB, LB)

        # dest layout for the batched is_equal: [128, NBLK * T]
        dest_t = np.ascontiguousarray(
            dest.reshape(NBLK * T, 128).T
        ).astype(np_dt)

        inv_t = np.zeros((BLK, NBLK), dtype=np.float32)
        iv = inv_cnt[c * NPC : (c + 1) * NPC]
        inv_t.T.flat[: NPC] = iv  # row-major [NBLK, BLK] view fill
        inv_t = np.ascontiguousarray(inv_t)

        in_maps.append(
            {
                "idxA": wA,
                "idxB": wB,
                "dest": dest_t,
                "invc": inv_t,
            }
        )

    shared = {
        "xcA": np.ascontiguousarray(np.asarray(x_clique)[:SPLIT]).astype(np_dt),
        "xcB": np.ascontiguousarray(np.asarray(x_clique)[SPLIT:]).astype(np_dt),
        "iota": np.tile(np.arange(128, dtype=np.float32), (128, 1)).astype(np_dt),
    }
    return in_maps, shared, T_A, T_B


# ----------------------------------------------------------------------------
# Kernel builder
# ----------------------------------------------------------------------------

def _build(T_A, T_B):
    T = T_A + T_B
    LA, LB = T_A * 128, T_B * 128
    NGRP = -(-NBLK // GRP)
    CB = N_CLIQUES - SPLIT

    from concourse.bacc import Bacc

    nc = Bacc(None)
    xcA = nc.declare_dram_parameter("xcA", [SPLIT, D], _DT, isOutput=False)
    xcB = nc.declare_dram_parameter("xcB", [CB, D], _DT, isOutput=False)
    idxA = nc.declare_dram_parameter(
        "idxA", [128, NGRP * GRP * LA // 16], mybir.dt.int16, isOutput=False
    )
    idxB = nc.declare_dram_parameter(
        "idxB", [128, NGRP * GRP * LB // 16], mybir.dt.int16, isOutput=False
    )
    dest = nc.declare_dram_parameter("dest", [128, NBLK * T], _DT, isOutput=False)
    invc = nc.declare_dram_parameter("invc", [128, NBLK], _F32, isOutput=False)
    iota = nc.declare_dram_parameter("iota", [128, 128], _DT, isOutput=False)
    wt = nc.declare_dram_parameter("wt", [128, 128], _DT, isOutput=False)
    bb = nc.declare_dram_parameter("bb", [128, 128], _F32, isOutput=False)
    out = nc.declare_dram_parameter("out", [NPAD, D], _F32, isOutput=True)

    from contextlib import ExitStack

    with PatchedTileContext(nc) as tc, ExitStack() as ctx:
        const = ctx.enter_context(tc.tile_pool(name="const", bufs=1))
        sb = ctx.enter_context(tc.tile_pool(name="sb", bufs=3))
        gpool = ctx.enter_context(tc.tile_pool(name="g", bufs=2))
        ps = ctx.enter_context(tc.tile_pool(name="ps", bufs=2, space="PSUM"))

        idxA_t = const.tile([128, NGRP * GRP * LA // 16], mybir.dt.int16)
        nc.sync.dma_start(idxA_t[:], idxA[:])
        idxB_t = const.tile([128, NGRP * GRP * LB // 16], mybir.dt.int16)
        nc.sync.dma_start(idxB_t[:], idxB[:])
        dest_t = const.tile([128, NBLK * T], _DT)
        nc.sync.dma_start(dest_t[:], dest[:])
        invc_t = const.tile([128, NBLK], _F32)
        nc.sync.dma_start(invc_t[:], invc[:])
        iota_t = const.tile([128, 128], _DT)
        nc.sync.dma_start(iota_t[:], iota[:])
        wt_t = const.tile([128, 128], _DT)
        nc.sync.dma_start(wt_t[:], wt[:])
        bb_t = const.tile([128, 128], _F32)
        nc.sync.dma_start(bb_t[:], bb[:])

        nA = GRP * LA
        nB = GRP * LB
        for g in range(NGRP):
            gA = gpool.tile([128, GRP * T_A, 128], _DT, tag="gA")
            nc.gpsimd.dma_gather(
                gA[:],
                xcA[:],
                idxA_t[:, g * (nA // 16) : (g + 1) * (nA // 16)],
                nA,
                nA,
                D,
                single_packet=False,
            )
            gB = gpool.tile([128, GRP * T_B, 128], _DT, tag="gB")
            nc.gpsimd.dma_gather(
                gB[:],
                xcB[:],
                idxB_t[:, g * (nB // 16) : (g + 1) * (nB // 16)],
                nB,
                nB,
                D,
                single_packet=False,
            )
            for i in range(GRP):
                b = g * GRP + i
                if b >= NBLK:
                    break
                onehot = sb.tile([128, T, 128], _DT, tag="oh")
                nc.vector.tensor_tensor(
                    out=onehot[:],
                    in0=dest_t[:, b * T : (b + 1) * T, None].to_broadcast(
                        [128, T, 128]
                    ),
                    in1=iota_t[:, None, :].to_broadcast([128, T, 128]),
                    op=mybir.AluOpType.is_equal,
                )
                # accum[f, n] += G[e, f].T @ onehot[e, n] — the gathered tile
                # must be the STATIONARY operand (LDWEIGHTS path); the moving
                # path crashes the PE when reading a dma_gather-written tile.
                accum = ps.tile([128, 128], _F32, tag="acc")
                for t in range(T_A):
                    nc.tensor.matmul(
                        out=accum[:],
                        lhsT=gA[:, i * T_A + t, :],
                        rhs=onehot[:, t, :],
                        start=(t == 0),
                        stop=False,
                    )
                for t in range(T_B):
                    nc.tensor.matmul(
                        out=accum[:],
                        lhsT=gB[:, i * T_B + t, :],
                        rhs=onehot[:, T_A + t, :],
                        start=False,
                        stop=(t == T_B - 1),
                    )
                # accum is summed.T — exactly the lhsT the Linear wants.
                acc_sb = sb.tile([128, 128], _DT, tag="accsb")
                nc.scalar.activation(
                    acc_sb[:], accum[:], mybir.ActivationFunctionType.Copy
                )
                lin = ps.tile([128, 128], _F32, tag="lin")
                nc.tensor.matmul(
                    out=lin[:], lhsT=acc_sb[:], rhs=wt_t[:], start=True, stop=True
                )
                # out[n, o] = lin[n, o] / count[n] + b[o]
                sc = sb.tile([128, 128], _F32, tag="sc")
                nc.scalar.activation(
                    sc[:],
                    lin[:],
                    mybir.ActivationFunctionType.Copy,
                    scale=invc_t[:, b : b + 1],
                )
                outs = sb.tile([128, 128], _F32, tag="outs")
                nc.vector.tensor_tensor(
                    out=outs[:], in0=sc[:], in1=bb_t[:], op=mybir.AluOpType.add
                )
                nc.sync.dma_start(out[b * 128 : (b + 1) * 128, :], outs[:])

    nc.finalize()
    return nc


_BUILD_CACHE = {}


def kernel(x, x_clique, node2clique_index, W, b, _trace=False, _tmpdir=None):
    in_maps, shared, T_A, T_B = _prepare(x_clique, node2clique_index)

    shared["wt"] = np.ascontiguousarray(np.asarray(W, dtype=np.float32).T).astype(
        _NP_DT
    )
    shared["bb"] = np.tile(
        np.asarray(b, dtype=np.float32)[None, :], (128, 1)
    ).astype(np.float32)

    key = (T_A, T_B, USE_BF16)
    if key not in _BUILD_CACHE:
        _BUILD_CACHE[key] = _build(T_A, T_B)
    nc = _BUILD_CACHE[key]

    full_maps = [dict(m, **shared) for m in in_maps]
    kwargs = {}
    if _trace:
        kwargs = dict(trace=True, tmpdir=_tmpdir)
    res = run_bass_kernel_spmd(nc, full_maps, core_ids=list(range(N_CORES)), **kwargs)

    out = np.concatenate(
        [res.results[c]["out"][:NPC] for c in range(N_CORES)], axis=0
    ).astype(np.float32)
    if _trace:
        return out, res
    return out



# revision 2
# speedup vs baseline: 2.8382x; 2.8382x over previous
"""Trainium2 Bass kernel for Clique2NodeConvBasic (GNN message passing).

Computes, for N=100000 nodes, C=50000 cliques, E=1600000 edges, D=128:

    gathered = x_clique[clique_idx]            # [E, 128]
    summed   = segment_sum(gathered, node_idx) # [N, 128]
    mean     = summed / max(count, 1)
    out      = mean @ W.T + b                  # [N, 128]

Sharding: edges partitioned by destination-node range across 8 NeuronCores
(12500 nodes per core); x_clique and the Linear weights replicated.

v2 design (from microbenchmarks on this hardware):
  - The bottleneck is GpSimd Q7 descriptor generation inside dma_gather
    (~7.9 ns per gathered row on one SWDGE queue pair). dma_gather
    instructions issued on DIFFERENT SWDGE queues (queue_num 0-3) execute
    on different Q7 core pairs and OVERLAP: 4-queue round-robin measured
    2.53 ns/row effective (3.2x).
  - All float data is bf16: halves the DMA drain (256B descriptors) and
    doubles PE matmul throughput. rel err ~1e-3, gate is 2e-2.
  - Tight packing: per (core, block) tile counts are data-dependent; the
    SPMD program is uniform across cores by padding each block position to
    the max tile count over the 8 cores (~+5% rows vs ~+12% for the old
    global-max padding).
  - Per-block accumulate in PSUM via one-hot matmuls (gathered tile is the
    STATIONARY operand; the moving-operand path crashes on dma_gather-
    written tiles), then Linear + 1/count scale + bias epilogue per block.
  - ap_gather / scatter_add / trailing -1 trimming / single_packet=True
    were all benched: ap_gather is 27.7 ns/idx, -1 trimming makes calls
    slower, single_packet=True hangs the device. Avoided.
"""

import os
import sys
import types

sys.path.insert(0, "/opt/trn_rl_repo")

import numpy as np

import concourse.bass as bass
import concourse.mybir as mybir
import concourse.tile as tile
from concourse.vector_clock import ScopedClock, VectorClock
from concourse.bass_utils import run_bass_kernel_spmd

# ----------------------------------------------------------------------------
# Environment shims
# ----------------------------------------------------------------------------

def _install_ntff_shim():
    """Register the axon NTFF profile hook if the image's antenv lacks it."""
    try:
        import antenv
    except ImportError:
        return
    if hasattr(antenv, "axon_hooks"):
        return
    hooks_mod = types.ModuleType("antenv.axon_hooks")
    _store = [None]
    hooks_mod.set_axon_ntff_profile_hook = lambda h: _store.__setitem__(0, h)
    hooks_mod.get_axon_ntff_profile_hook = lambda: _store[0]
    sys.modules["antenv.axon_hooks"] = hooks_mod
    antenv.axon_hooks = hooks_mod
    try:
        from trn_agent_boot.trn_boot import _ntff_profile_via_ctypes

        hook = _ntff_profile_via_ctypes("/opt/axon/libaxon_pjrt.so")
        if hook is not None:
            hooks_mod.set_axon_ntff_profile_hook(hook)
    except Exception:
        pass


_install_ntff_shim()


class PatchedTileContext(tile.TileContext):
    """Spread the tail-drain's sem waits over a chain of SP NOPs.

    The walrus build in this container caps sync-waits per instruction
    (setupSyncWait: "Too many sync wait commands"), while stock Tile
    attaches every outstanding proc's wait to one Drain. One NOP per
    proc keeps every instruction at a single wait.
    """

    def _drain_and_barrier(self, tick_clock, wait_clock):
        gc = tick_clock.global_clock
        for p, t in enumerate(gc):
            if t <= 0:
                continue
            nop = self.nc.sync.nop()
            part = VectorClock()
            part.require_at_least(p, t)
            wait_clock.add_sem_waits(nop.ins, ScopedClock({None: part}))
        self.nc.sync.drain()
        self.nc.all_engine_barrier()
        assert self.sems is not None
        popped = self.nc._tile_sem_poison_stack.pop()
        assert popped is self._sem_poison
        self.nc.clear_and_free_semaphores(list(self.sems.allocated().values()))
        self.nc.all_engine_barrier()


# ----------------------------------------------------------------------------
# Problem constants (hardcoded per the task contract)
# ----------------------------------------------------------------------------

N_NODES = 100000
N_CLIQUES = 50000
D = 128
N_CORES = 8
NPC = N_NODES // N_CORES        # 12500 nodes per core
BLK = 128                       # destination nodes per block
NBLK = -(-NPC // BLK)           # 98 blocks per core (last partial: 84)
NPAD = NBLK * BLK               # 12544 padded output rows per core
SPLIT = 32768                   # int16-index limit for dma_gather
NT = 32                         # 128-row tiles per dma_gather call
NQ = 4                          # SWDGE queues used round-robin
PAD_DEST = -1000.0              # one-hot miss value for padding slots

_F32 = mybir.dt.float32
_BF16 = mybir.dt.bfloat16

import ml_dtypes

_NP_BF16 = np.dtype(ml_dtypes.bfloat16)


# ----------------------------------------------------------------------------
# Host-side preparation
# ----------------------------------------------------------------------------

def _prepare(x_clique, node2clique_index):
    """Sort/bucket the edge list. Returns per-core input dicts plus the
    (data-dependent) uniform schedule."""
    node = np.asarray(node2clique_index[0]).astype(np.int64)
    clique = np.asarray(node2clique_index[1]).astype(np.int64)

    counts = np.bincount(node, minlength=N_NODES).astype(np.float64)
    inv_cnt = (1.0 / np.maximum(counts, 1.0)).astype(np.float32)

    order = np.argsort(node, kind="stable")
    ns = node[order]
    cs = clique[order]
    core_bounds = np.searchsorted(ns, np.arange(N_CORES + 1) * NPC)

    # Per-core stable partition: (block, is_b) groups, A before B.
    per_core = []
    cntA = np.zeros((N_CORES, NBLK), dtype=np.int64)
    cntB = np.zeros((N_CORES, NBLK), dtype=np.int64)
    for c in range(N_CORES):
        lo, hi = core_bounds[c], core_bounds[c + 1]
        loc = ns[lo:hi] - c * NPC
        cq = cs[lo:hi]
        blk = loc // BLK
        win = loc % BLK
        is_b = cq >= SPLIT
        key = blk * 2 + is_b
        sub = np.argsort(key, kind="stable")
        blk, win, cq, is_b = blk[sub], win[sub], cq[sub], is_b[sub]
        cntA[c] = np.bincount(blk[~is_b], minlength=NBLK)
        cntB[c] = np.bincount(blk[is_b], minlength=NBLK)
        per_core.append((blk, win, cq, is_b))

    # Uniform schedule: per block position, max tile count over cores.
    tAmax = -(-cntA.max(axis=0) // 128)           # [NBLK]
    tBmax = np.maximum(-(-cntB.max(axis=0) // 128), 1)
    offA = np.concatenate([[0], np.cumsum(tAmax)])  # tile offset per block
    offB = np.concatenate([[0], np.cumsum(tBmax)])
    totA, totB = int(offA[-1]), int(offB[-1])
    callsA, callsB = -(-totA // NT), -(-totB // NT)
    padA, padB = callsA * NT, callsB * NT

    def _wrap(idx):
        # [L] -> [128, L/16] (16-partition wrap, replicated to 8 core groups)
        w = idx.reshape(-1, 16).T.copy().reshape(16, -1)
        return np.tile(w, (8, 1))

    in_maps = []
    for c in range(N_CORES):
        blk, win, cq, is_b = per_core[c]
        idxA = np.zeros(padA * 128, dtype=np.int16)
        idxB = np.zeros(padB * 128, dtype=np.int16)
        destA = np.full(padA * 128, PAD_DEST, dtype=np.float32)
        destB = np.full(padB * 128, PAD_DEST, dtype=np.float32)

        a = ~is_b
        posA = np.arange(a.sum()) - np.concatenate([[0], np.cumsum(cntA[c])])[blk[a]]
        posB = np.arange(is_b.sum()) - np.concatenate([[0], np.cumsum(cntB[c])])[blk[is_b]]
        slotA = offA[blk[a]] * 128 + posA
        slotB = offB[blk[is_b]] * 128 + posB
        idxA[slotA] = cq[a].astype(np.int16)
        idxB[slotB] = (cq[is_b] - SPLIT).astype(np.int16)
        destA[slotA] = win[a]
        destB[slotB] = win[is_b]

        # dest layout for is_equal: [128 slot, n_tiles]
        destA_t = np.ascontiguousarray(destA.reshape(padA, 128).T).astype(_NP_BF16)
        destB_t = np.ascontiguousarray(destB.reshape(padB, 128).T).astype(_NP_BF16)

        inv_t = np.zeros((BLK, NBLK), dtype=np.float32)
        inv_t.T.flat[:NPC] = inv_cnt[c * NPC : (c + 1) * NPC]

        in_maps.append(
            {
                "idxA": _wrap(idxA),
                "idxB": _wrap(idxB),
                "destA": destA_t,
                "destB": destB_t,
                "invc": np.ascontiguousarray(inv_t),
            }
        )

    xc = np.asarray(x_clique)
    shared = {
        "xcA": np.ascontiguousarray(xc[:SPLIT]).astype(_NP_BF16),
        "xcB": np.ascontiguousarray(xc[SPLIT:]).astype(_NP_BF16),
        "iota": np.tile(np.arange(128, dtype=np.float32), (128, 1)).astype(_NP_BF16),
    }
    sched = (tuple(int(t) for t in tAmax), tuple(int(t) for t in tBmax))
    return in_maps, shared, sched


# ----------------------------------------------------------------------------
# Kernel builder
# ----------------------------------------------------------------------------

def _build(sched):
    tAmax, tBmax = np.array(sched[0]), np.array(sched[1])
    offA = np.concatenate([[0], np.cumsum(tAmax)])
    offB = np.concatenate([[0], np.cumsum(tBmax)])
    totA, totB = int(offA[-1]), int(offB[-1])
    callsA, callsB = -(-totA // NT), -(-totB // NT)
    padA, padB = callsA * NT, callsB * NT
    CB = N_CLIQUES - SPLIT

    from concourse.bacc import Bacc

    nc = Bacc(None, num_swdge_queues=NQ)
    xcA = nc.declare_dram_parameter("xcA", [SPLIT, D], _BF16, isOutput=False)
    xcB = nc.declare_dram_parameter("xcB", [CB, D], _BF16, isOutput=False)
    idxA = nc.declare_dram_parameter("idxA", [128, padA * 8], mybir.dt.int16, isOutput=False)
    idxB = nc.declare_dram_parameter("idxB", [128, padB * 8], mybir.dt.int16, isOutput=False)
    destA = nc.declare_dram_parameter("destA", [128, padA], _BF16, isOutput=False)
    destB = nc.declare_dram_parameter("destB", [128, padB], _BF16, isOutput=False)
    invc = nc.declare_dram_parameter("invc", [128, NBLK], _F32, isOutput=False)
    iota = nc.declare_dram_parameter("iota", [128, 128], _BF16, isOutput=False)
    wt = nc.declare_dram_parameter("wt", [128, 128], _BF16, isOutput=False)
    bb = nc.declare_dram_parameter("bb", [128, 128], _F32, isOutput=False)
    out = nc.declare_dram_parameter("out", [NPAD, D], _F32, isOutput=True)

    # merged gather-call order: by first block each call serves (A first on tie)
    def start_block(off, k):
        return int(np.searchsorted(off, k * NT, side="right") - 1)

    merged = sorted(
        [(start_block(offA, k), 0, k) for k in range(callsA)]
        + [(start_block(offB, k), 1, k) for k in range(callsB)],
        key=lambda t: (t[0], t[1], t[2]),
    )

    from contextlib import ExitStack

    with PatchedTileContext(nc) as tc, ExitStack() as ctx:
        const = ctx.enter_context(tc.tile_pool(name="const", bufs=1))
        gpool = ctx.enter_context(tc.tile_pool(name="g", bufs=2))
        opool = ctx.enter_context(tc.tile_pool(name="o", bufs=2))
        sb = ctx.enter_context(tc.tile_pool(name="sb", bufs=2))
        ps = ctx.enter_context(tc.tile_pool(name="ps", bufs=4, space="PSUM"))
        psl = ctx.enter_context(tc.tile_pool(name="psl", bufs=2, space="PSUM"))

        idxA_t = const.tile([128, padA * 8], mybir.dt.int16)
        nc.sync.dma_start(idxA_t[:], idxA[:])
        idxB_t = const.tile([128, padB * 8], mybir.dt.int16)
        nc.sync.dma_start(idxB_t[:], idxB[:])
        destA_t = const.tile([128, padA], _BF16)
        nc.sync.dma_start(destA_t[:], destA[:])
        destB_t = const.tile([128, padB], _BF16)
        nc.sync.dma_start(destB_t[:], destB[:])
        invc_t = const.tile([128, NBLK], _F32)
        nc.sync.dma_start(invc_t[:], invc[:])
        iota_t = const.tile([128, 128], _BF16)
        nc.sync.dma_start(iota_t[:], iota[:])
        wt_t = const.tile([128, 128], _BF16)
        nc.sync.dma_start(wt_t[:], wt[:])
        bb_t = const.tile([128, 128], _F32)
        nc.sync.dma_start(bb_t[:], bb[:])

        call_tiles = {}   # (stream, k) -> (gathered tile, onehot tile)
        emitted = [0]

        def emit_calls(up_to_block):
            while emitted[0] < len(merged) and merged[emitted[0]][0] <= up_to_block:
                _, stream, k = merged[emitted[0]]
                qi = emitted[0] % NQ
                src = xcA if stream == 0 else xcB
                idx_t = idxA_t if stream == 0 else idxB_t
                dest_t = destA_t if stream == 0 else destB_t
                g_t = gpool.tile([128, NT, 128], _BF16, tag=f"g{qi}")
                nc.gpsimd.dma_gather(
                    g_t[:],
                    src[:],
                    idx_t[:, k * NT * 8 : (k + 1) * NT * 8],
                    NT * 128,
                    NT * 128,
                    D,
                    single_packet=False,
                    queue_num=qi,
                )
                oh_t = opool.tile([128, NT, 128], _BF16, tag=f"o{qi}")
                nc.vector.tensor_tensor(
                    out=oh_t[:],
                    in0=dest_t[:, k * NT : (k + 1) * NT, None].to_broadcast(
                        [128, NT, 128]
                    ),
                    in1=iota_t[:, None, :].to_broadcast([128, NT, 128]),
                    op=mybir.AluOpType.is_equal,
                )
                call_tiles[(stream, k)] = (g_t, oh_t)
                emitted[0] += 1

        for b in range(NBLK):
            emit_calls(b)
            mms = [(0, t) for t in range(int(offA[b]), int(offA[b + 1]))] + [
                (1, t) for t in range(int(offB[b]), int(offB[b + 1]))
            ]
            accum = ps.tile([128, 128], _F32, tag="acc")
            for i, (stream, t) in enumerate(mms):
                g_t, oh_t = call_tiles[(stream, t // NT)]
                slot = t % NT
                nc.tensor.matmul(
                    out=accum[:],
                    lhsT=g_t[:, slot, :],
                    rhs=oh_t[:, slot, :],
                    start=(i == 0),
                    stop=(i == len(mms) - 1),
                )
            # accum[f, n] is summed.T — exactly the lhsT the Linear wants.
            acc_sb = sb.tile([128, 128], _BF16, tag="accsb")
            nc.scalar.activation(
                acc_sb[:], accum[:], mybir.ActivationFunctionType.Copy
            )
            lin = psl.tile([128, 128], _F32, tag="lin")
            nc.tensor.matmul(
                out=lin[:], lhsT=acc_sb[:], rhs=wt_t[:], start=True, stop=True
            )
            # out[n, o] = lin[n, o] / count[n] + b[o]
            sc = sb.tile([128, 128], _F32, tag="sc")
            nc.scalar.activation(
                sc[:],
                lin[:],
                mybir.ActivationFunctionType.Copy,
                scale=invc_t[:, b : b + 1],
            )
            outs = sb.tile([128, 128], _F32, tag="outs")
            nc.vector.tensor_tensor(
                out=outs[:], in0=sc[:], in1=bb_t[:], op=mybir.AluOpType.add
            )
            nc.sync.dma_start(out[b * 128 : (b + 1) * 128, :], outs[:])

    nc.finalize()
    return nc


_BUILD_CACHE = {}


def kernel(x, x_clique, node2clique_index, W, b, _trace=False, _tmpdir=None):
    in_maps, shared, sched = _prepare(x_clique, node2clique_index)

    shared["wt"] = np.ascontiguousarray(
        np.asarray(W, dtype=np.float32).T
    ).astype(_NP_BF16)
    shared["bb"] = np.tile(
        np.asarray(b, dtype=np.float32)[None, :], (128, 1)
    ).astype(np.float32)

    if sched not in _BUILD_CACHE:
        _BUILD_CACHE[sched] = _build(sched)
    nc = _BUILD_CACHE[sched]

    full_maps = [dict(m, **shared) for m in in_maps]
    kwargs = {}
    if _trace:
        kwargs = dict(trace=True, tmpdir=_tmpdir)
    res = run_bass_kernel_spmd(nc, full_maps, core_ids=list(range(N_CORES)), **kwargs)

    out = np.concatenate(
        [res.results[c]["out"][:NPC] for c in range(N_CORES)], axis=0
    ).astype(np.float32)
    if _trace:
        return out, res
    return out


# revision 10
# speedup vs baseline: 3.5327x; 1.2447x over previous
"""Trainium2 Bass kernel for Clique2NodeConvBasic (GNN message passing).

Computes, for N=100000 nodes, C=50000 cliques, E=1600000 edges, D=128:

    gathered = x_clique[clique_idx]            # [E, 128]
    summed   = segment_sum(gathered, node_idx) # [N, 128]
    mean     = summed / max(count, 1)
    out      = mean @ W.T + b                  # [N, 128]

Sharding: edges partitioned by destination-node range across 8 NeuronCores
(12500 nodes per core); x_clique and the Linear weights replicated.

v2 design (from microbenchmarks on this hardware):
  - The bottleneck is GpSimd Q7 descriptor generation inside dma_gather
    (~7.9 ns per gathered row on one SWDGE queue pair). dma_gather
    instructions issued on DIFFERENT SWDGE queues (queue_num 0-3) execute
    on different Q7 core pairs and OVERLAP: 4-queue round-robin measured
    2.53 ns/row effective (3.2x).
  - All float data is bf16: halves the DMA drain (256B descriptors) and
    doubles PE matmul throughput. rel err ~1e-3, gate is 2e-2.
  - Tight packing: per (core, block) tile counts are data-dependent; the
    SPMD program is uniform across cores by padding each block position to
    the max tile count over the 8 cores (~+5% rows vs ~+12% for the old
    global-max padding).
  - Per-block accumulate in PSUM via one-hot matmuls (gathered tile is the
    STATIONARY operand; the moving-operand path crashes on dma_gather-
    written tiles), then Linear + 1/count scale + bias epilogue per block.
  - ap_gather / scatter_add / trailing -1 trimming / single_packet=True
    were all benched: ap_gather is 27.7 ns/idx, -1 trimming makes calls
    slower, single_packet=True hangs the device. Avoided.
"""

import os
import sys
import types

sys.path.insert(0, "/opt/trn_rl_repo")

import numpy as np

import concourse.bass as bass
import concourse.mybir as mybir
import concourse.tile as tile
from concourse.vector_clock import ScopedClock, VectorClock
from concourse.bass_utils import run_bass_kernel_spmd

# ----------------------------------------------------------------------------
# Environment shims
# ----------------------------------------------------------------------------

def _install_ntff_shim():
    """Register the axon NTFF profile hook if the image's antenv lacks it."""
    try:
        import antenv
    except ImportError:
        return
    if hasattr(antenv, "axon_hooks"):
        return
    hooks_mod = types.ModuleType("antenv.axon_hooks")
    _store = [None]
    hooks_mod.set_axon_ntff_profile_hook = lambda h: _store.__setitem__(0, h)
    hooks_mod.get_axon_ntff_profile_hook = lambda: _store[0]
    sys.modules["antenv.axon_hooks"] = hooks_mod
    antenv.axon_hooks = hooks_mod
    try:
        from trn_agent_boot.trn_boot import _ntff_profile_via_ctypes

        hook = _ntff_profile_via_ctypes("/opt/axon/libaxon_pjrt.so")
        if hook is not None:
            hooks_mod.set_axon_ntff_profile_hook(hook)
    except Exception:
        pass


_install_ntff_shim()


class PatchedTileContext(tile.TileContext):
    """Spread the tail-drain's sem waits over a chain of SP NOPs.

    The walrus build in this container caps sync-waits per instruction
    (setupSyncWait: "Too many sync wait commands"), while stock Tile
    attaches every outstanding proc's wait to one Drain. One NOP per
    proc keeps every instruction at a single wait.
    """

    def _drain_and_barrier(self, tick_clock, wait_clock):
        gc = tick_clock.global_clock
        for p, t in enumerate(gc):
            if t <= 0:
                continue
            nop = self.nc.sync.nop()
            part = VectorClock()
            part.require_at_least(p, t)
            wait_clock.add_sem_waits(nop.ins, ScopedClock({None: part}))
        self.nc.sync.drain()
        self.nc.all_engine_barrier()
        assert self.sems is not None
        popped = self.nc._tile_sem_poison_stack.pop()
        assert popped is self._sem_poison
        self.nc.clear_and_free_semaphores(list(self.sems.allocated().values()))
        self.nc.all_engine_barrier()


# ----------------------------------------------------------------------------
# Problem constants (hardcoded per the task contract)
# ----------------------------------------------------------------------------

N_NODES = 100000
N_CLIQUES = 50000
D = 128
N_CORES = 8
NPC = N_NODES // N_CORES        # 12500 nodes per core
BLK = 128                       # destination nodes per block
NBLK = -(-NPC // BLK)           # 98 blocks per core (last partial: 84)
NPAD = NBLK * BLK               # 12544 padded output rows per core
SPLIT = 32768                   # int16-index limit for dma_gather
NT = 32                         # 128-row tiles per dma_gather call
NQ = 4                          # SWDGE queues used round-robin
PAD_DEST = -1000.0              # one-hot miss value for padding slots

_F32 = mybir.dt.float32
_BF16 = mybir.dt.bfloat16

import ml_dtypes

_NP_BF16 = np.dtype(ml_dtypes.bfloat16)


# ----------------------------------------------------------------------------
# Host-side preparation
# ----------------------------------------------------------------------------

def _prepare(x_clique, node2clique_index):
    """Sort/bucket the edge list. Returns per-core input dicts plus the
    (data-dependent) uniform schedule."""
    node = np.asarray(node2clique_index[0]).astype(np.int64)
    clique = np.asarray(node2clique_index[1]).astype(np.int64)

    counts = np.bincount(node, minlength=N_NODES).astype(np.float64)
    inv_cnt = (1.0 / np.maximum(counts, 1.0)).astype(np.float32)

    order = np.argsort(node, kind="stable")
    ns = node[order]
    cs = clique[order]
    core_bounds = np.searchsorted(ns, np.arange(N_CORES + 1) * NPC)

    # Per-core stable partition: (block, is_b) groups, A before B.
    per_core = []
    cntA = np.zeros((N_CORES, NBLK), dtype=np.int64)
    cntB = np.zeros((N_CORES, NBLK), dtype=np.int64)
    for c in range(N_CORES):
        lo, hi = core_bounds[c], core_bounds[c + 1]
        loc = ns[lo:hi] - c * NPC
        cq = cs[lo:hi]
        blk = loc // BLK
        win = loc % BLK
        is_b = cq >= SPLIT
        key = blk * 2 + is_b
        sub = np.argsort(key, kind="stable")
        blk, win, cq, is_b = blk[sub], win[sub], cq[sub], is_b[sub]
        cntA[c] = np.bincount(blk[~is_b], minlength=NBLK)
        cntB[c] = np.bincount(blk[is_b], minlength=NBLK)
        per_core.append((blk, win, cq, is_b))

    # Uniform schedule: per block position, max tile count over cores.
    tAmax = -(-cntA.max(axis=0) // 128)           # [NBLK]
    tBmax = np.maximum(-(-cntB.max(axis=0) // 128), 1)
    offA = np.concatenate([[0], np.cumsum(tAmax)])  # tile offset per block
    offB = np.concatenate([[0], np.cumsum(tBmax)])
    totA, totB = int(offA[-1]), int(offB[-1])
    callsA, callsB = -(-totA // NT), -(-totB // NT)
    padA, padB = callsA * NT, callsB * NT

    def _wrap(idx):
        # [L] -> [128, L/16] (16-partition wrap, replicated to 8 core groups)
        w = idx.reshape(-1, 16).T.copy().reshape(16, -1)
        return np.tile(w, (8, 1))

    in_maps = []
    for c in range(N_CORES):
        blk, win, cq, is_b = per_core[c]
        idxA = np.zeros(padA * 128, dtype=np.int16)
        idxB = np.zeros(padB * 128, dtype=np.int16)
        destA = np.full(padA * 128, PAD_DEST, dtype=np.float32)
        destB = np.full(padB * 128, PAD_DEST, dtype=np.float32)

        a = ~is_b
        posA = np.arange(a.sum()) - np.concatenate([[0], np.cumsum(cntA[c])])[blk[a]]
        posB = np.arange(is_b.sum()) - np.concatenate([[0], np.cumsum(cntB[c])])[blk[is_b]]
        slotA = offA[blk[a]] * 128 + posA
        slotB = offB[blk[is_b]] * 128 + posB
        idxA[slotA] = cq[a].astype(np.int16)
        idxB[slotB] = (cq[is_b] - SPLIT).astype(np.int16)
        destA[slotA] = win[a]
        destB[slotB] = win[is_b]

        # dest layout for is_equal: [128 slot, n_tiles]
        destA_t = np.ascontiguousarray(destA.reshape(padA, 128).T).astype(_NP_BF16)
        destB_t = np.ascontiguousarray(destB.reshape(padB, 128).T).astype(_NP_BF16)

        inv_t = np.zeros((BLK, NBLK), dtype=np.float32)
        inv_t.T.flat[:NPC] = inv_cnt[c * NPC : (c + 1) * NPC]

        # max(cnt,1) so zero-count nodes still get +bias after the 1/max(cnt,1)
        # scale: (0 + 1*b)*1 = b, matching segment-mean-with-clamp semantics.
        cnt_row = np.zeros((1, NPAD), dtype=np.float32)
        cnt_row[0, :NPC] = np.maximum(counts[c * NPC : (c + 1) * NPC], 1.0)

        in_maps.append(
            {
                "idxA": _wrap(idxA),
                "idxB": _wrap(idxB),
                "destA": destA_t,
                "destB": destB_t,
                "invc": np.ascontiguousarray(inv_t),
                "cntb": cnt_row.astype(_NP_BF16),
            }
        )

    xc = np.asarray(x_clique)
    shared = {
        "xcA": np.ascontiguousarray(xc[:SPLIT]).astype(_NP_BF16),
        "xcB": np.ascontiguousarray(xc[SPLIT:]).astype(_NP_BF16),
        "iota": np.tile(np.arange(128, dtype=np.float32), (128, 1)).astype(_NP_BF16),
    }
    sched = (tuple(int(t) for t in tAmax), tuple(int(t) for t in tBmax))
    return in_maps, shared, sched


# ----------------------------------------------------------------------------
# Kernel builder
# ----------------------------------------------------------------------------

def _build(sched):
    tAmax, tBmax = np.array(sched[0]), np.array(sched[1])
    offA = np.concatenate([[0], np.cumsum(tAmax)])
    offB = np.concatenate([[0], np.cumsum(tBmax)])
    totA, totB = int(offA[-1]), int(offB[-1])
    callsA, callsB = -(-totA // NT), -(-totB // NT)
    padA, padB = callsA * NT, callsB * NT
    CB = N_CLIQUES - SPLIT

    from concourse.bacc import Bacc

    nc = Bacc(None, num_swdge_queues=NQ)
    xcA = nc.declare_dram_parameter("xcA", [SPLIT, D], _BF16, isOutput=False)
    xcB = nc.declare_dram_parameter("xcB", [CB, D], _BF16, isOutput=False)
    idxA = nc.declare_dram_parameter("idxA", [128, padA * 8], mybir.dt.int16, isOutput=False)
    idxB = nc.declare_dram_parameter("idxB", [128, padB * 8], mybir.dt.int16, isOutput=False)
    destA = nc.declare_dram_parameter("destA", [128, padA], _BF16, isOutput=False)
    destB = nc.declare_dram_parameter("destB", [128, padB], _BF16, isOutput=False)
    invc = nc.declare_dram_parameter("invc", [128, NBLK], _F32, isOutput=False)
    iota = nc.declare_dram_parameter("iota", [128, 128], _BF16, isOutput=False)
    wt = nc.declare_dram_parameter("wt", [128, 128], _BF16, isOutput=False)
    brow = nc.declare_dram_parameter("brow", [1, 128], _BF16, isOutput=False)
    cntb = nc.declare_dram_parameter("cntb", [1, NPAD], _BF16, isOutput=False)
    out = nc.declare_dram_parameter("out", [NPAD, D], _F32, isOutput=True)

    # Gather-call lists per stream: full NT-tile calls, but the final call is
    # split into <=8-tile sub-calls so the tail blocks' data lands sooner.
    def call_list(tot):
        calls = []
        t = 0
        while t < tot:
            nt = NT if tot - t > NT else min(8, tot - t)
            calls.append((t, nt))
            t += nt
        return calls

    callsA_l = call_list(totA)
    callsB_l = call_list(totB)

    # merged gather-call order: by first block each call serves (A first on tie)
    def start_block(off, t0):
        return int(np.searchsorted(off, t0, side="right") - 1)

    merged = sorted(
        [(start_block(offA, t0), 0, i) for i, (t0, nt) in enumerate(callsA_l)]
        + [(start_block(offB, t0), 1, i) for i, (t0, nt) in enumerate(callsB_l)],
        key=lambda t: (t[0], t[1], t[2]),
    )

    # tile index -> (call index, slot) per stream
    def tile_map(calls):
        m = {}
        for i, (t0, nt) in enumerate(calls):
            for s in range(nt):
                m[t0 + s] = (i, s)
        return m

    tmapA = tile_map(callsA_l)
    tmapB = tile_map(callsB_l)

    from contextlib import ExitStack

    with PatchedTileContext(nc) as tc, ExitStack() as ctx:
        const = ctx.enter_context(tc.tile_pool(name="const", bufs=1))
        gpool = ctx.enter_context(tc.tile_pool(name="g", bufs=2))
        opool = ctx.enter_context(tc.tile_pool(name="o", bufs=2))
        sb = ctx.enter_context(tc.tile_pool(name="sb", bufs=2))
        ps = ctx.enter_context(tc.tile_pool(name="ps", bufs=4, space="PSUM"))
        psl = ctx.enter_context(tc.tile_pool(name="psl", bufs=2, space="PSUM"))

        # idx consts loaded in chunks so the first gathers wait only on the
        # slices they read, not the whole index upload.
        idxA_t = const.tile([128, padA * 8], mybir.dt.int16)
        for lo in range(0, padA * 8, NT * 8):
            hi = min(lo + NT * 8, padA * 8)
            nc.sync.dma_start(idxA_t[:, lo:hi], idxA[:, lo:hi])
        idxB_t = const.tile([128, padB * 8], mybir.dt.int16)
        for lo in range(0, padB * 8, NT * 8):
            hi = min(lo + NT * 8, padB * 8)
            nc.sync.dma_start(idxB_t[:, lo:hi], idxB[:, lo:hi])
        destA_t = const.tile([128, padA], _BF16)
        nc.sync.dma_start(destA_t[:], destA[:])
        destB_t = const.tile([128, padB], _BF16)
        nc.sync.dma_start(destB_t[:], destB[:])
        invc_t = const.tile([128, NBLK], _F32)
        nc.sync.dma_start(invc_t[:], invc[:])
        iota_t = const.tile([128, 128], _BF16)
        nc.sync.dma_start(iota_t[:], iota[:])
        wt_t = const.tile([128, 128], _BF16)
        nc.sync.dma_start(wt_t[:], wt[:])
        brow_t = const.tile([1, 128], _BF16)
        nc.sync.dma_start(brow_t[:], brow[:])
        cntb_t = const.tile([1, NPAD], _BF16)
        nc.sync.dma_start(cntb_t[:], cntb[:])

        call_tiles = {}   # (stream, k) -> (gathered tile, onehot tile)
        emitted = [0]

        def emit_calls(up_to_block):
            while emitted[0] < len(merged) and merged[emitted[0]][0] <= up_to_block:
                _, stream, k = merged[emitted[0]]
                qi = emitted[0] % NQ
                src = xcA if stream == 0 else xcB
                idx_t = idxA_t if stream == 0 else idxB_t
                dest_t = destA_t if stream == 0 else destB_t
                t0, nt = (callsA_l if stream == 0 else callsB_l)[k]
                g_t = gpool.tile([128, NT, 128], _BF16, tag=f"g{qi}")
                nc.gpsimd.dma_gather(
                    g_t[:, :nt, :],
                    src[:],
                    idx_t[:, t0 * 8 : (t0 + nt) * 8],
                    nt * 128,
                    nt * 128,
                    D,
                    single_packet=False,
                    queue_num=qi,
                )
                oh_t = opool.tile([128, NT, 128], _BF16, tag=f"o{qi}")
                nc.vector.tensor_tensor(
                    out=oh_t[:, :nt, :],
                    in0=dest_t[:, t0 : t0 + nt, None].to_broadcast(
                        [128, nt, 128]
                    ),
                    in1=iota_t[:, None, :].to_broadcast([128, nt, 128]),
                    op=mybir.AluOpType.is_equal,
                )
                call_tiles[(stream, k)] = (g_t, oh_t)
                emitted[0] += 1

        for b in range(NBLK):
            emit_calls(b)
            mms = [(0, t) for t in range(int(offA[b]), int(offA[b + 1]))] + [
                (1, t) for t in range(int(offB[b]), int(offB[b + 1]))
            ]
            accum = ps.tile([128, 128], _F32, tag="acc")
            for i, (stream, t) in enumerate(mms):
                ci, slot = (tmapA if stream == 0 else tmapB)[t]
                g_t, oh_t = call_tiles[(stream, ci)]
                nc.tensor.matmul(
                    out=accum[:],
                    lhsT=g_t[:, slot, :],
                    rhs=oh_t[:, slot, :],
                    start=(i == 0),
                    stop=(i == len(mms) - 1),
                )
            # accum[f, n] is summed.T — exactly the lhsT the Linear wants.
            acc_sb = sb.tile([128, 128], _BF16, tag="accsb")
            nc.scalar.activation(
                acc_sb[:], accum[:], mybir.ActivationFunctionType.Copy
            )
            # lin[n, o] = summed[n, :] @ W.T + max(cnt[n],1)*b[o]; the rank-1
            # count*bias term makes the later 1/max(cnt,1) scale yield "+b".
            lin = psl.tile([128, 128], _F32, tag="lin")
            nc.tensor.matmul(
                out=lin[:], lhsT=acc_sb[:], rhs=wt_t[:], start=True, stop=False
            )
            nc.tensor.matmul(
                out=lin[:],
                lhsT=cntb_t[:, b * 128 : (b + 1) * 128],
                rhs=brow_t[:],
                start=False,
                stop=True,
            )
            # out[n, o] = lin[n, o] / max(count[n], 1)
            sc = sb.tile([128, 128], _F32, tag="sc")
            nc.scalar.activation(
                sc[:],
                lin[:],
                mybir.ActivationFunctionType.Copy,
                scale=invc_t[:, b : b + 1],
            )
            nc.sync.dma_start(out[b * 128 : (b + 1) * 128, :], sc[:])

    nc.finalize()
    return nc


_BUILD_CACHE = {}


def kernel(x, x_clique, node2clique_index, W, b, _trace=False, _tmpdir=None):
    in_maps, shared, sched = _prepare(x_clique, node2clique_index)

    shared["wt"] = np.ascontiguousarray(
        np.asarray(W, dtype=np.float32).T
    ).astype(_NP_BF16)
    shared["brow"] = np.asarray(b, dtype=np.float32)[None, :].astype(_NP_BF16)

    if sched not in _BUILD_CACHE:
        _BUILD_CACHE[sched] = _build(sched)
    nc = _BUILD_CACHE[sched]

    full_maps = [dict(m, **shared) for m in in_maps]
    kwargs = {}
    if _trace:
        kwargs = dict(trace=True, tmpdir=_tmpdir)
    res = run_bass_kernel_spmd(nc, full_maps, core_ids=list(range(N_CORES)), **kwargs)

    out = np.concatenate(
        [res.results[c]["out"][:NPC] for c in range(N_CORES)], axis=0
    ).astype(np.float32)
    if _trace:
        return out, res
    return out


# revision 11
# speedup vs baseline: 3.5660x; 1.0094x over previous
"""Trainium2 Bass kernel for Clique2NodeConvBasic (GNN message passing).

Computes, for N=100000 nodes, C=50000 cliques, E=1600000 edges, D=128:

    gathered = x_clique[clique_idx]            # [E, 128]
    summed   = segment_sum(gathered, node_idx) # [N, 128]
    mean     = summed / max(count, 1)
    out      = mean @ W.T + b                  # [N, 128]

Sharding: edges partitioned by destination-node range across 8 NeuronCores
(12500 nodes per core); x_clique and the Linear weights replicated.

v2 design (from microbenchmarks on this hardware):
  - The bottleneck is GpSimd Q7 descriptor generation inside dma_gather
    (~7.9 ns per gathered row on one SWDGE queue pair). dma_gather
    instructions issued on DIFFERENT SWDGE queues (queue_num 0-3) execute
    on different Q7 core pairs and OVERLAP: 4-queue round-robin measured
    2.53 ns/row effective (3.2x).
  - All float data is bf16: halves the DMA drain (256B descriptors) and
    doubles PE matmul throughput. rel err ~1e-3, gate is 2e-2.
  - Tight packing: per (core, block) tile counts are data-dependent; the
    SPMD program is uniform across cores by padding each block position to
    the max tile count over the 8 cores (~+5% rows vs ~+12% for the old
    global-max padding).
  - Per-block accumulate in PSUM via one-hot matmuls (gathered tile is the
    STATIONARY operand; the moving-operand path crashes on dma_gather-
    written tiles), then Linear + 1/count scale + bias epilogue per block.
  - ap_gather / scatter_add / trailing -1 trimming / single_packet=True
    were all benched: ap_gather is 27.7 ns/idx, -1 trimming makes calls
    slower, single_packet=True hangs the device. Avoided.
"""

import os
import sys
import types

sys.path.insert(0, "/opt/trn_rl_repo")

import numpy as np

import concourse.bass as bass
import concourse.mybir as mybir
import concourse.tile as tile
from concourse.vector_clock import ScopedClock, VectorClock
from concourse.bass_utils import run_bass_kernel_spmd

# ----------------------------------------------------------------------------
# Environment shims
# ----------------------------------------------------------------------------

def _install_ntff_shim():
    """Register the axon NTFF profile hook if the image's antenv lacks it."""
    try:
        import antenv
    except ImportError:
        return
    if hasattr(antenv, "axon_hooks"):
        return
    hooks_mod = types.ModuleType("antenv.axon_hooks")
    _store = [None]
    hooks_mod.set_axon_ntff_profile_hook = lambda h: _store.__setitem__(0, h)
    hooks_mod.get_axon_ntff_profile_hook = lambda: _store[0]
    sys.modules["antenv.axon_hooks"] = hooks_mod
    antenv.axon_hooks = hooks_mod
    try:
        from trn_agent_boot.trn_boot import _ntff_profile_via_ctypes

        hook = _ntff_profile_via_ctypes("/opt/axon/libaxon_pjrt.so")
        if hook is not None:
            hooks_mod.set_axon_ntff_profile_hook(hook)
    except Exception:
        pass


_install_ntff_shim()


class PatchedTileContext(tile.TileContext):
    """Spread the tail-drain's sem waits over a chain of SP NOPs.

    The walrus build in this container caps sync-waits per instruction
    (setupSyncWait: "Too many sync wait commands"), while stock Tile
    attaches every outstanding proc's wait to one Drain. One NOP per
    proc keeps every instruction at a single wait.
    """

    def _drain_and_barrier(self, tick_clock, wait_clock):
        gc = tick_clock.global_clock
        for p, t in enumerate(gc):
            if t <= 0:
                continue
            nop = self.nc.sync.nop()
            part = VectorClock()
            part.require_at_least(p, t)
            wait_clock.add_sem_waits(nop.ins, ScopedClock({None: part}))
        self.nc.sync.drain()
        self.nc.all_engine_barrier()
        assert self.sems is not None
        popped = self.nc._tile_sem_poison_stack.pop()
        assert popped is self._sem_poison
        self.nc.clear_and_free_semaphores(list(self.sems.allocated().values()))
        self.nc.all_engine_barrier()


# ----------------------------------------------------------------------------
# Problem constants (hardcoded per the task contract)
# ----------------------------------------------------------------------------

N_NODES = 100000
N_CLIQUES = 50000
D = 128
N_CORES = 8
NPC = N_NODES // N_CORES        # 12500 nodes per core
BLK = 128                       # destination nodes per block
NBLK = -(-NPC // BLK)           # 98 blocks per core (last partial: 84)
NPAD = NBLK * BLK               # 12544 padded output rows per core
SPLIT = 32768                   # int16-index limit for dma_gather
NT = 24                         # 128-row tiles per dma_gather call
NQ = 4                          # SWDGE queues used round-robin
PAD_DEST = -1000.0              # one-hot miss value for padding slots

_F32 = mybir.dt.float32
_BF16 = mybir.dt.bfloat16

import ml_dtypes

_NP_BF16 = np.dtype(ml_dtypes.bfloat16)


# ----------------------------------------------------------------------------
# Host-side preparation
# ----------------------------------------------------------------------------

def _prepare(x_clique, node2clique_index):
    """Sort/bucket the edge list. Returns per-core input dicts plus the
    (data-dependent) uniform schedule."""
    node = np.asarray(node2clique_index[0]).astype(np.int64)
    clique = np.asarray(node2clique_index[1]).astype(np.int64)

    counts = np.bincount(node, minlength=N_NODES).astype(np.float64)
    inv_cnt = (1.0 / np.maximum(counts, 1.0)).astype(np.float32)

    order = np.argsort(node, kind="stable")
    ns = node[order]
    cs = clique[order]
    core_bounds = np.searchsorted(ns, np.arange(N_CORES + 1) * NPC)

    # Per-core stable partition: (block, is_b) groups, A before B.
    per_core = []
    cntA = np.zeros((N_CORES, NBLK), dtype=np.int64)
    cntB = np.zeros((N_CORES, NBLK), dtype=np.int64)
    for c in range(N_CORES):
        lo, hi = core_bounds[c], core_bounds[c + 1]
        loc = ns[lo:hi] - c * NPC
        cq = cs[lo:hi]
        blk = loc // BLK
        win = loc % BLK
        is_b = cq >= SPLIT
        key = blk * 2 + is_b
        sub = np.argsort(key, kind="stable")
        blk, win, cq, is_b = blk[sub], win[sub], cq[sub], is_b[sub]
        cntA[c] = np.bincount(blk[~is_b], minlength=NBLK)
        cntB[c] = np.bincount(blk[is_b], minlength=NBLK)
        per_core.append((blk, win, cq, is_b))

    # Uniform schedule: per block position, max tile count over cores.
    tAmax = -(-cntA.max(axis=0) // 128)           # [NBLK]
    tBmax = np.maximum(-(-cntB.max(axis=0) // 128), 1)
    offA = np.concatenate([[0], np.cumsum(tAmax)])  # tile offset per block
    offB = np.concatenate([[0], np.cumsum(tBmax)])
    totA, totB = int(offA[-1]), int(offB[-1])
    callsA, callsB = -(-totA // NT), -(-totB // NT)
    padA, padB = callsA * NT, callsB * NT

    def _wrap(idx):
        # [L] -> [128, L/16] (16-partition wrap, replicated to 8 core groups)
        w = idx.reshape(-1, 16).T.copy().reshape(16, -1)
        return np.tile(w, (8, 1))

    in_maps = []
    for c in range(N_CORES):
        blk, win, cq, is_b = per_core[c]
        idxA = np.zeros(padA * 128, dtype=np.int16)
        idxB = np.zeros(padB * 128, dtype=np.int16)
        destA = np.full(padA * 128, PAD_DEST, dtype=np.float32)
        destB = np.full(padB * 128, PAD_DEST, dtype=np.float32)

        a = ~is_b
        posA = np.arange(a.sum()) - np.concatenate([[0], np.cumsum(cntA[c])])[blk[a]]
        posB = np.arange(is_b.sum()) - np.concatenate([[0], np.cumsum(cntB[c])])[blk[is_b]]
        slotA = offA[blk[a]] * 128 + posA
        slotB = offB[blk[is_b]] * 128 + posB
        idxA[slotA] = cq[a].astype(np.int16)
        idxB[slotB] = (cq[is_b] - SPLIT).astype(np.int16)
        destA[slotA] = win[a]
        destB[slotB] = win[is_b]

        # dest layout for is_equal: [128 slot, n_tiles]
        destA_t = np.ascontiguousarray(destA.reshape(padA, 128).T).astype(_NP_BF16)
        destB_t = np.ascontiguousarray(destB.reshape(padB, 128).T).astype(_NP_BF16)

        inv_t = np.zeros((BLK, NBLK), dtype=np.float32)
        inv_t.T.flat[:NPC] = inv_cnt[c * NPC : (c + 1) * NPC]

        # max(cnt,1) so zero-count nodes still get +bias after the 1/max(cnt,1)
        # scale: (0 + 1*b)*1 = b, matching segment-mean-with-clamp semantics.
        cnt_row = np.zeros((1, NPAD), dtype=np.float32)
        cnt_row[0, :NPC] = np.maximum(counts[c * NPC : (c + 1) * NPC], 1.0)

        in_maps.append(
            {
                "idxA": _wrap(idxA),
                "idxB": _wrap(idxB),
                "destA": destA_t,
                "destB": destB_t,
                "invc": np.ascontiguousarray(inv_t),
                "cntb": cnt_row.astype(_NP_BF16),
            }
        )

    xc = np.asarray(x_clique)
    shared = {
        "xcA": np.ascontiguousarray(xc[:SPLIT]).astype(_NP_BF16),
        "xcB": np.ascontiguousarray(xc[SPLIT:]).astype(_NP_BF16),
        "iota": np.tile(np.arange(128, dtype=np.float32), (128, 1)).astype(_NP_BF16),
    }
    sched = (tuple(int(t) for t in tAmax), tuple(int(t) for t in tBmax))
    return in_maps, shared, sched


# ----------------------------------------------------------------------------
# Kernel builder
# ----------------------------------------------------------------------------

def _build(sched):
    tAmax, tBmax = np.array(sched[0]), np.array(sched[1])
    offA = np.concatenate([[0], np.cumsum(tAmax)])
    offB = np.concatenate([[0], np.cumsum(tBmax)])
    totA, totB = int(offA[-1]), int(offB[-1])
    callsA, callsB = -(-totA // NT), -(-totB // NT)
    padA, padB = callsA * NT, callsB * NT
    CB = N_CLIQUES - SPLIT

    from concourse.bacc import Bacc

    nc = Bacc(None, num_swdge_queues=NQ)
    xcA = nc.declare_dram_parameter("xcA", [SPLIT, D], _BF16, isOutput=False)
    xcB = nc.declare_dram_parameter("xcB", [CB, D], _BF16, isOutput=False)
    idxA = nc.declare_dram_parameter("idxA", [128, padA * 8], mybir.dt.int16, isOutput=False)
    idxB = nc.declare_dram_parameter("idxB", [128, padB * 8], mybir.dt.int16, isOutput=False)
    destA = nc.declare_dram_parameter("destA", [128, padA], _BF16, isOutput=False)
    destB = nc.declare_dram_parameter("destB", [128, padB], _BF16, isOutput=False)
    invc = nc.declare_dram_parameter("invc", [128, NBLK], _F32, isOutput=False)
    iota = nc.declare_dram_parameter("iota", [128, 128], _BF16, isOutput=False)
    wt = nc.declare_dram_parameter("wt", [128, 128], _BF16, isOutput=False)
    brow = nc.declare_dram_parameter("brow", [1, 128], _BF16, isOutput=False)
    cntb = nc.declare_dram_parameter("cntb", [1, NPAD], _BF16, isOutput=False)
    out = nc.declare_dram_parameter("out", [NPAD, D], _F32, isOutput=True)

    # Gather-call lists per stream: full NT-tile calls, but the final call is
    # split into <=8-tile sub-calls so the tail blocks' data lands sooner.
    def call_list(tot):
        calls = []
        t = 0
        while t < tot:
            nt = NT if tot - t > NT else min(8, tot - t)
            calls.append((t, nt))
            t += nt
        return calls

    callsA_l = call_list(totA)
    callsB_l = call_list(totB)

    # merged gather-call order: by first block each call serves (A first on tie)
    def start_block(off, t0):
        return int(np.searchsorted(off, t0, side="right") - 1)

    merged = sorted(
        [(start_block(offA, t0), 0, i) for i, (t0, nt) in enumerate(callsA_l)]
        + [(start_block(offB, t0), 1, i) for i, (t0, nt) in enumerate(callsB_l)],
        key=lambda t: (t[0], t[1], t[2]),
    )

    # tile index -> (call index, slot) per stream
    def tile_map(calls):
        m = {}
        for i, (t0, nt) in enumerate(calls):
            for s in range(nt):
                m[t0 + s] = (i, s)
        return m

    tmapA = tile_map(callsA_l)
    tmapB = tile_map(callsB_l)

    from contextlib import ExitStack

    with PatchedTileContext(nc) as tc, ExitStack() as ctx:
        const = ctx.enter_context(tc.tile_pool(name="const", bufs=1))
        gpool = ctx.enter_context(tc.tile_pool(name="g", bufs=3))
        opool = ctx.enter_context(tc.tile_pool(name="o", bufs=3))
        sb = ctx.enter_context(tc.tile_pool(name="sb", bufs=2))
        ps = ctx.enter_context(tc.tile_pool(name="ps", bufs=4, space="PSUM"))
        psl = ctx.enter_context(tc.tile_pool(name="psl", bufs=2, space="PSUM"))

        # idx consts loaded in chunks so the first gathers wait only on the
        # slices they read, not the whole index upload.
        idxA_t = const.tile([128, padA * 8], mybir.dt.int16)
        for lo in range(0, padA * 8, NT * 8):
            hi = min(lo + NT * 8, padA * 8)
            nc.sync.dma_start(idxA_t[:, lo:hi], idxA[:, lo:hi])
        idxB_t = const.tile([128, padB * 8], mybir.dt.int16)
        for lo in range(0, padB * 8, NT * 8):
            hi = min(lo + NT * 8, padB * 8)
            nc.sync.dma_start(idxB_t[:, lo:hi], idxB[:, lo:hi])
        destA_t = const.tile([128, padA], _BF16)
        nc.sync.dma_start(destA_t[:], destA[:])
        destB_t = const.tile([128, padB], _BF16)
        nc.sync.dma_start(destB_t[:], destB[:])
        invc_t = const.tile([128, NBLK], _F32)
        nc.sync.dma_start(invc_t[:], invc[:])
        iota_t = const.tile([128, 128], _BF16)
        nc.sync.dma_start(iota_t[:], iota[:])
        wt_t = const.tile([128, 128], _BF16)
        nc.sync.dma_start(wt_t[:], wt[:])
        brow_t = const.tile([1, 128], _BF16)
        nc.sync.dma_start(brow_t[:], brow[:])
        cntb_t = const.tile([1, NPAD], _BF16)
        nc.sync.dma_start(cntb_t[:], cntb[:])

        call_tiles = {}   # (stream, k) -> (gathered tile, onehot tile)
        emitted = [0]

        def emit_calls(up_to_block):
            while emitted[0] < len(merged) and merged[emitted[0]][0] <= up_to_block:
                _, stream, k = merged[emitted[0]]
                qi = emitted[0] % NQ
                src = xcA if stream == 0 else xcB
                idx_t = idxA_t if stream == 0 else idxB_t
                dest_t = destA_t if stream == 0 else destB_t
                t0, nt = (callsA_l if stream == 0 else callsB_l)[k]
                g_t = gpool.tile([128, NT, 128], _BF16, tag=f"g{qi}")
                nc.gpsimd.dma_gather(
                    g_t[:, :nt, :],
                    src[:],
                    idx_t[:, t0 * 8 : (t0 + nt) * 8],
                    nt * 128,
                    nt * 128,
                    D,
                    single_packet=False,
                    queue_num=qi,
                )
                oh_t = opool.tile([128, NT, 128], _BF16, tag=f"o{qi}")
                nc.vector.tensor_tensor(
                    out=oh_t[:, :nt, :],
                    in0=dest_t[:, t0 : t0 + nt, None].to_broadcast(
                        [128, nt, 128]
                    ),
                    in1=iota_t[:, None, :].to_broadcast([128, nt, 128]),
                    op=mybir.AluOpType.is_equal,
                )
                call_tiles[(stream, k)] = (g_t, oh_t)
                emitted[0] += 1

        for b in range(NBLK):
            emit_calls(b)
            mms = [(0, t) for t in range(int(offA[b]), int(offA[b + 1]))] + [
                (1, t) for t in range(int(offB[b]), int(offB[b + 1]))
            ]
            accum = ps.tile([128, 128], _F32, tag="acc")
            for i, (stream, t) in enumerate(mms):
                ci, slot = (tmapA if stream == 0 else tmapB)[t]
                g_t, oh_t = call_tiles[(stream, ci)]
                nc.tensor.matmul(
                    out=accum[:],
                    lhsT=g_t[:, slot, :],
                    rhs=oh_t[:, slot, :],
                    start=(i == 0),
                    stop=(i == len(mms) - 1),
                )
            # accum[f, n] is summed.T — exactly the lhsT the Linear wants.
            acc_sb = sb.tile([128, 128], _BF16, tag="accsb")
            nc.scalar.activation(
                acc_sb[:], accum[:], mybir.ActivationFunctionType.Copy
            )
            # lin[n, o] = summed[n, :] @ W.T + max(cnt[n],1)*b[o]; the rank-1
            # count*bias term makes the later 1/max(cnt,1) scale yield "+b".
            lin = psl.tile([128, 128], _F32, tag="lin")
            nc.tensor.matmul(
                out=lin[:], lhsT=acc_sb[:], rhs=wt_t[:], start=True, stop=False
            )
            nc.tensor.matmul(
                out=lin[:],
                lhsT=cntb_t[:, b * 128 : (b + 1) * 128],
                rhs=brow_t[:],
                start=False,
                stop=True,
            )
            # out[n, o] = lin[n, o] / max(count[n], 1)
            sc = sb.tile([128, 128], _F32, tag="sc")
            nc.scalar.activation(
                sc[:],
                lin[:],
                mybir.ActivationFunctionType.Copy,
                scale=invc_t[:, b : b + 1],
            )
            nc.sync.dma_start(out[b * 128 : (b + 1) * 128, :], sc[:])

    nc.finalize()
    return nc


_BUILD_CACHE = {}


def kernel(x, x_clique, node2clique_index, W, b, _trace=False, _tmpdir=None):
    in_maps, shared, sched = _prepare(x_clique, node2clique_index)

    shared["wt"] = np.ascontiguousarray(
        np.asarray(W, dtype=np.float32).T
    ).astype(_NP_BF16)
    shared["brow"] = np.asarray(b, dtype=np.float32)[None, :].astype(_NP_BF16)

    if sched not in _BUILD_CACHE:
        _BUILD_CACHE[sched] = _build(sched)
    nc = _BUILD_CACHE[sched]

    full_maps = [dict(m, **shared) for m in in_maps]
    kwargs = {}
    if _trace:
        kwargs = dict(trace=True, tmpdir=_tmpdir)
    res = run_bass_kernel_spmd(nc, full_maps, core_ids=list(range(N_CORES)), **kwargs)

    out = np.concatenate(
        [res.results[c]["out"][:NPC] for c in range(N_CORES)], axis=0
    ).astype(np.float32)
    if _trace:
        return out, res
    return out
